# revision 19
# baseline (speedup 1.0000x reference)
"""Trainium2 Bass kernel for nn_MultiHeadAttention_37039797961289.

MHA: B=1, S=4096, D=768, H=12, HD=64, fp32 in/out.

Sharding (v6): sequence split into 8 slices of 512. Core c:
  - projects Q^T and, for head-pairs 2-5, K^T/V for ITS OWN 512-token
    slice (bf16); 4 chunked AllGathers (kt_m + v_m packed) stream
    those pairs' K^T/V blocks to every core
  - head-pairs 0 and 1 are projected REDUNDANTLY over the FULL
    sequence on every core: the first collective is gated by the
    slowest core's launch (~80-100us inter-core skew in this runner,
    jittering run to run), so the first two attention pairs run
    entirely from local SBUF while the collectives ride out the skew —
    real work instead of idling (which also down-clocks the PE)
  - flash-style attention for its 512 queries over all 4096 keys
  - output-projects its 512 rows (bf16); host concatenates.

Layout choices:
  - Q^T/K^T kept transposed [feat, seq]; scores contraction over HD=64.
  - V stored with a per-head ones-column (65-wide head groups): one
    DMA (or SBUF slice) per (head-pair, key-block) provides the attn
    stationary including the softmax-denominator column (PSUM row 64).
  - bf16 on all matmul paths (same PE speed as fp32r, half the
    DMA/collective bytes, ~3e-3 rel err); fp32 PSUM + softmax math.
  - scores^T tiles are [t=128, sq=512]; exp split between ACT (direct
    from PSUM) and DVE-evac + big-chunk ACT to balance engines.
  - per-head-pair epilogue: DVE evacuates attn PSUM to SBUF right
    after the drain, then reciprocal + gpsimd partition-broadcast +
    DVE multiply off the critical path. The LAST pair instead uses a
    matmul-with-ones partition broadcast of the reciprocal (no DMA
    hops) and phase D splits its contraction in two 64-row halves so
    no cross-partition move ever blocks the output projection.
  - phases C and D share one pool scope; phase D's PSUM tiles reuse
    the scores-pool slots so no pool-exit barrier splits the phases.
"""

import sys

sys.path.insert(0, "/opt/trn_rl_repo")

import ml_dtypes
import numpy as np

import concourse.bass as bass
import concourse.mybir as mybir
import concourse.tile as tile
from concourse import bacc
from concourse.bass_utils import run_bass_kernel_spmd

FP32 = mybir.dt.float32
FP32R = mybir.dt.float32r
BF16 = mybir.dt.bfloat16
EXP = mybir.ActivationFunctionType.Exp
IDENT = mybir.ActivationFunctionType.Identity

N_CORES = 8
D = 768
H = 12
HD = 64
S = 4096
SQ = S // N_CORES  # 512 queries/keys per core
KC = D // 128  # 6 contraction chunks of 128 over D
MP = 6  # 6 head-pair chunks of 128 rows in Q^T/K^T
NREP = 2  # head-pairs 0..NREP-1 are replicated, the rest allgathered
PW = 2 * (HD + 1)  # 130: V row width per head pair incl. ones cols
KTN = 128 * SQ  # 65536 elems: kt_m chunk in the packed AG buffer
AGN = KTN + SQ * PW  # 132096 elems per packed (kt_m, v_m) AG input


def build_nc():
    nc = bacc.Bacc(None)

    xct = nc.dram_tensor("xct", [128, KC, SQ], BF16, kind="ExternalInput")
    xt = nc.dram_tensor("xt", [128, KC, S], BF16, kind="ExternalInput")
    wq = nc.dram_tensor("wq", [128, KC, D], BF16, kind="ExternalInput")
    wk = nc.dram_tensor("wk", [128, KC, D], BF16, kind="ExternalInput")
    wv = nc.dram_tensor("wv", [128, KC, D], BF16, kind="ExternalInput")
    wo = nc.dram_tensor("wo", [128, MP, D], BF16, kind="ExternalInput")
    bq = nc.dram_tensor("bq", [128, MP], FP32, kind="ExternalInput")
    bk = nc.dram_tensor("bk", [128, MP], FP32, kind="ExternalInput")
    bv = nc.dram_tensor("bv", [1, D], FP32, kind="ExternalInput")
    bo = nc.dram_tensor("bo", [1, D], FP32, kind="ExternalInput")
    out = nc.dram_tensor("out", [SQ, D], FP32, kind="ExternalOutput")

    with tile.TileContext(nc) as tc:
        with tc.tile_pool(name="persist", bufs=1) as persist, \
             tc.tile_pool(name="dram", bufs=1, space="DRAM") as dpool:

            ag_in = {
                m: dpool.tile([1, AGN], BF16, name=f"ag_in{m}")
                for m in range(NREP, MP)
            }
            ag_out = {
                m: dpool.tile([N_CORES, AGN], BF16, name=f"ag_out{m}",
                              addr_space="Shared")
                for m in range(NREP, MP)
            }

            def allgather(m):
                nc.gpsimd.collective_compute(
                    "AllGather",
                    mybir.AluOpType.bypass,
                    replica_groups=[list(range(N_CORES))],
                    ins=[ag_in[m].opt()],
                    outs=[ag_out[m].opt()],
                )

            # ---- resident tiles ----
            bk_sb = persist.tile([128, MP], FP32, name="bk_sb")
            nc.scalar.dma_start(bk_sb[:], bk[:])
            bv_sb = persist.tile([1, D], FP32, name="bv_sb")
            nc.scalar.dma_start(bv_sb[:], bv[:])
            bv_bc = persist.tile([128, D], FP32, name="bv_bc")
            bq_sb = persist.tile([128, MP], FP32, name="bq_sb")
            nc.scalar.dma_start(bq_sb[:], bq[:])
            bo_sb = persist.tile([1, D], FP32, name="bo_sb")
            bo_bc = persist.tile([128, D], FP32, name="bo_bc")
            ones_f = persist.tile([HD + 1, HD], BF16, name="ones_f")
            nc.vector.memset(ones_f[:], 1.0)

            # Q^T m-chunks stay resident for all of phase C
            qt_sb = [
                persist.tile([128, SQ], BF16, name=f"qt_{m}") for m in range(MP)
            ]
            # normalized attn^T per head pair, resident until phase D
            attp_sb = [
                persist.tile([128, SQ], BF16, name=f"attp_{m}")
                for m in range(MP - 1)
            ]
            att5u = persist.tile([HD, SQ], BF16, name="att5u")
            att5l = persist.tile([HD, SQ], BF16, name="att5l")
            wo_sb = persist.tile([128, MP, D], BF16, name="wo_sb")
            wo5_lo = persist.tile([HD, D], BF16, name="wo5_lo")

            # replicated pairs 0..NREP-1: full-sequence K^T and V
            ktf = [
                persist.tile([128, S], BF16, name=f"ktf_{m}")
                for m in range(NREP)
            ]
            # [seq-part, seq-chunk, pair, head, 65]
            vf = persist.tile([128, S // 128, NREP, 2, HD + 1], BF16, name="vf")
            nc.vector.memset(vf[:, :, :, :, HD : HD + 1], 1.0)

            # ---- phase A ----
            with tc.tile_pool(name="wpool", bufs=1) as wpool, \
                 tc.tile_pool(name="psA", bufs=2, space="PSUM") as psA:
                xct_sb = wpool.tile([128, KC, SQ], BF16, name="xct_sb")
                wk_sb = wpool.tile([128, KC, D], BF16, name="wk_sb")
                wv_sb = wpool.tile([128, KC, D], BF16, name="wv_sb")
                wq_sb = wpool.tile([128, KC, D], BF16, name="wq_sb")
                xt_sb = wpool.tile([128, KC, S], BF16, name="xt_sb")
                # own-slice inputs first (pairs 2-5 weight columns),
                # then xt for the replicated pairs; wq/wo land last
                nc.sync.dma_start(xct_sb[:, 0, :], xct[:, 0, :])
                nc.gpsimd.dma_start(wk_sb[:, 0, 256:768], wk[:, 0, 256:768])
                nc.scalar.dma_start(wv_sb[:, :, 256:768], wv[:, :, 256:768])
                nc.sync.dma_start(xct_sb[:, 1:KC, :], xct[:, 1:KC, :])
                for _k in range(1, KC):
                    nc.gpsimd.dma_start(wk_sb[:, _k, 256:768], wk[:, _k, 256:768])
                nc.gpsimd.partition_broadcast(bv_bc[:], bv_sb[:])
                nc.scalar.dma_start(wq_sb[:], wq[:])
                nc.sync.dma_start(xt_sb[:, :, 0:1536], xt[:, :, 0:1536])
                nc.gpsimd.dma_start(wk_sb[:, :, 0:256], wk[:, :, 0:256])
                nc.gpsimd.dma_start(xt_sb[:, :, 1536:3072], xt[:, :, 1536:3072])
                nc.scalar.dma_start(xt_sb[:, :, 3072:4096], xt[:, :, 3072:4096])

                kt_sb = wpool.tile([128, MP, SQ], BF16, name="kt_sb")
                v_own = wpool.tile([128, 4, H, HD + 1], BF16, name="v_own")
                nc.vector.memset(v_own[:, :, 4:12, HD : HD + 1], 1.0)

                def k_proj(m):
                    ps = psA.tile([128, SQ], FP32, name="proj_ps")
                    for k in range(KC):
                        nc.tensor.matmul(
                            ps[:],
                            wk_sb[:, k, 128 * m : 128 * (m + 1)],
                            xct_sb[:, k, :],
                            start=(k == 0),
                            stop=(k == KC - 1),
                        )
                    nc.scalar.activation(
                        kt_sb[:, m, :], ps[:], IDENT, bias=bk_sb[:, m : m + 1]
                    )
                    nc.sync.dma_start(
                        ag_in[m][0:1, 0:KTN].rearrange(
                            "o (p f) -> p (o f)", p=128
                        ),
                        kt_sb[:, m, :],
                    )

                def v_in_dma(m):
                    nc.sync.dma_start(
                        ag_in[m][0:1, KTN:AGN].rearrange(
                            "o (oo p h w) -> p (o oo) h w", oo=4, p=128, h=2
                        ),
                        v_own[:, :, 2 * m : 2 * m + 2, :],
                    )

                # pair 2 first (its AG leads the chain): K chunks + V heads 4-5
                k_proj(2)
                k_proj(3)
                for mt in range(4):
                    ps = psA.tile([128, 128], FP32, name="vp2_ps")
                    for k in range(KC):
                        nc.tensor.matmul(
                            ps[:],
                            xct_sb[:, k, 128 * mt : 128 * (mt + 1)],
                            wv_sb[:, k, 256:384],
                            start=(k == 0),
                            stop=(k == KC - 1),
                        )
                    nc.vector.tensor_add(
                        out=v_own[:, mt, 4:6, 0:HD],
                        in0=ps[:].rearrange("p (h w) -> p h w", h=2),
                        in1=bv_bc[:, 256:384].rearrange("p (h w) -> p h w", h=2),
                    )
                v_in_dma(2)
                allgather(2)

                # remaining loads (off the early critical path)
                nc.scalar.dma_start(wv_sb[:, :, 0:256], wv[:, :, 0:256])
                nc.scalar.dma_start(wo_sb[:], wo[:])
                nc.scalar.dma_start(bo_sb[:], bo[:])
                nc.gpsimd.partition_broadcast(bo_bc[:], bo_sb[:])
                nc.gpsimd.dma_start(wo5_lo[:], wo_sb[64:128, 5, :])

                # K chunks 4..5, V heads 6-11, AGs 3..5
                for m in range(4, MP):
                    k_proj(m)
                for mt in range(4):
                    ps = psA.tile([128, 384], FP32, name="v1_ps")
                    for k in range(KC):
                        nc.tensor.matmul(
                            ps[:],
                            xct_sb[:, k, 128 * mt : 128 * (mt + 1)],
                            wv_sb[:, k, 384:768],
                            start=(k == 0),
                            stop=(k == KC - 1),
                        )
                    nc.vector.tensor_add(
                        out=v_own[:, mt, 6:12, 0:HD],
                        in0=ps[:].rearrange("p (h w) -> p h w", h=6),
                        in1=bv_bc[:, 384:768].rearrange("p (h w) -> p h w", h=6),
                    )
                for m in range(3, MP):
                    v_in_dma(m)
                    allgather(m)

                # Q^T for own slice — fills the PE while xt streams in
                for m in range(MP):
                    ps = psA.tile([128, SQ], FP32, name="proj_ps")
                    for k in range(KC):
                        nc.tensor.matmul(
                            ps[:],
                            wq_sb[:, k, 128 * m : 128 * (m + 1)],
                            xct_sb[:, k, :],
                            start=(k == 0),
                            stop=(k == KC - 1),
                        )
                    nc.scalar.activation(
                        qt_sb[m][:], ps[:], IDENT, bias=bq_sb[:, m : m + 1]
                    )

                # replicated pairs: full-sequence K^T
                for m in range(NREP):
                    for b in range(N_CORES):
                        ps = psA.tile([128, SQ], FP32, name="proj_ps")
                        for k in range(KC):
                            nc.tensor.matmul(
                                ps[:],
                                wk_sb[:, k, 128 * m : 128 * (m + 1)],
                                xt_sb[:, k, SQ * b : SQ * (b + 1)],
                                start=(k == 0),
                                stop=(k == KC - 1),
                            )
                        nc.scalar.activation(
                            ktf[m][:, SQ * b : SQ * (b + 1)],
                            ps[:],
                            IDENT,
                            bias=bk_sb[:, m : m + 1],
                        )
                # replicated pairs: full-sequence V (both pairs per matmul)
                for c in range(S // 128):
                    ps = psA.tile([128, 128 * NREP], FP32, name="vf_ps")
                    for k in range(KC):
                        nc.tensor.matmul(
                            ps[:],
                            xt_sb[:, k, 128 * c : 128 * (c + 1)],
                            wv_sb[:, k, 0 : 128 * NREP],
                            start=(k == 0),
                            stop=(k == KC - 1),
                        )
                    nc.vector.tensor_add(
                        out=vf[:, c, :, :, 0:HD],
                        in0=ps[:].rearrange("p (pr h w) -> p pr h w", pr=NREP, h=2),
                        in1=bv_bc[:, 0 : 128 * NREP].rearrange(
                            "p (pr h w) -> p pr h w", pr=NREP, h=2
                        ),
                    )


            # ---- phases C+D: attention + output projection (one scope) ----
            with tc.tile_pool(name="ktg_pool", bufs=2) as ktg_pool, \
                 tc.tile_pool(name="vg_pool", bufs=2) as vg_pool, \
                 tc.tile_pool(name="exp_pool", bufs=2) as exp_pool, \
                 tc.tile_pool(name="ex_pool", bufs=4) as ex_pool, \
                 tc.tile_pool(name="sm_pool", bufs=2) as sm_pool, \
                 tc.tile_pool(name="opool", bufs=3) as opool, \
                 tc.tile_pool(name="pt_ps", bufs=3, space="PSUM") as pt_psp, \
                 tc.tile_pool(name="at_ps", bufs=1, space="PSUM") as at_psp:

                for m in range(MP):
                    if m >= NREP:
                        # bulk-stage the whole gathered pair into SBUF:
                        # two DMAs replace 16 per-iteration loads
                        ktg_t = ktg_pool.tile([128, N_CORES, SQ], BF16, name="ktg")
                        nc.sync.dma_start(
                            ktg_t[:],
                            ag_out[m][:, 0:KTN].rearrange(
                                "r (p f) -> p r f", p=128
                            ),
                        )
                        vg_t = vg_pool.tile(
                            [128, N_CORES, 4, PW], BF16, name="vg"
                        )
                        for _r in range(N_CORES):
                            nc.gpsimd.dma_start(
                                vg_t[:, _r, :, :],
                                ag_out[m][_r : _r + 1, KTN:AGN].rearrange(
                                    "o (oo p f) -> p (o oo) f", oo=4, p=128
                                ),
                            )
                    at0 = at_psp.tile([HD + 1, SQ], FP32, name="at0")
                    at1 = at_psp.tile([HD + 1, SQ], FP32, name="at1")
                    prev = None
                    for r in range(N_CORES):
                        if m < NREP:
                            ktt = ktf[m][:, SQ * r : SQ * (r + 1)]

                            def vsl_ap(j, hh, _m=m, _r=r):
                                return vf[:, 4 * _r + j, _m, hh, :]
                        else:
                            ktt = ktg_t[:, r, :]

                            def vsl_ap(j, hh, _vg=vg_t, _r=r):
                                return _vg[:, _r, j, (HD + 1) * hh : (HD + 1) * (hh + 1)]

                        # scores (PE); h0 exp via DVE evac + one big ACT,
                        # h1 exp directly from PSUM on ACT
                        sc0 = exp_pool.tile([128, 4, 512], FP32, name="sc_0")
                        ex1 = ex_pool.tile([128, 4, 512], BF16, name="ex1")
                        for blk in range(2):
                            pts = [
                                pt_psp.tile([128, 1024], FP32, name="pt")
                                for _ in range(2)
                            ]
                            for jj in range(2):
                                j = 2 * blk + jj
                                for hh in range(2):
                                    prange = slice(64 * hh, 64 * (hh + 1))
                                    nc.tensor.matmul(
                                        pts[hh][:, 512 * jj : 512 * (jj + 1)],
                                        ktt[prange, 128 * j : 128 * (j + 1)],
                                        qt_sb[m][prange, :],
                                        start=True,
                                        stop=True,
                                    )
                            nc.vector.tensor_copy(
                                out=sc0[:, 2 * blk : 2 * blk + 2, :],
                                in_=pts[0][:],
                            )
                            nc.scalar.activation(
                                ex1[:, 2 * blk : 2 * blk + 2, :], pts[1][:], EXP
                            )

                        ex0 = ex_pool.tile([128, 4, 512], BF16, name="ex0")
                        nc.scalar.activation(ex0[:], sc0[:], EXP)
                        exs = [ex0, ex1]

                        # attention matmuls for the PREVIOUS r (exp done)
                        if prev is not None:
                            pexs, pvsl, pr = prev
                            for hh in range(2):
                                att_ps = at0 if hh == 0 else at1
                                for j in range(4):
                                    nc.tensor.matmul(
                                        att_ps[:],
                                        pvsl(j, hh),
                                        pexs[hh][:, j, :],
                                        start=(pr == 0 and j == 0),
                                        stop=False,
                                    )
                        prev = (exs, vsl_ap, r)

                    # drain: attention for the last r
                    pexs, pvsl, pr = prev
                    for hh in range(2):
                        att_ps = at0 if hh == 0 else at1
                        for j in range(4):
                            nc.tensor.matmul(
                                att_ps[:],
                                pvsl(j, hh),
                                pexs[hh][:, j, :],
                                start=False,
                                stop=(j == 3),
                            )

                    if m < MP - 1:
                        # evacuate attn PSUM to SBUF fast (frees banks for
                        # m+1); normalize off the critical path
                        raw0 = sm_pool.tile([HD + 1, SQ], FP32, name="raw0")
                        nc.vector.tensor_copy(out=raw0[:], in_=at0[:])
                        raw1 = sm_pool.tile([HD + 1, SQ], FP32, name="raw1")
                        nc.vector.tensor_copy(out=raw1[:], in_=at1[:])

                        dn2 = sm_pool.tile([2, SQ], FP32, name="dn2")
                        nc.gpsimd.dma_start(dn2[0:1, :], raw0[HD : HD + 1, :])
                        nc.gpsimd.dma_start(dn2[1:2, :], raw1[HD : HD + 1, :])
                        rec2 = sm_pool.tile([2, SQ], FP32, name="rec2")
                        nc.vector.reciprocal(rec2[:], dn2[:])
                        rec1b = sm_pool.tile([1, SQ], FP32, name="rec1b")
                        nc.gpsimd.dma_start(rec1b[:], rec2[1:2, :])
                        bc0 = sm_pool.tile([HD, SQ], FP32, name="bc0")
                        nc.gpsimd.partition_broadcast(bc0[:], rec2[0:1, :])
                        nc.vector.tensor_mul(
                            out=attp_sb[m][0:HD, :],
                            in0=raw0[0:HD, :],
                            in1=bc0[:],
                        )
                        bc1 = sm_pool.tile([HD, SQ], FP32, name="bc1")
                        nc.gpsimd.partition_broadcast(bc1[:], rec1b[:])
                        a1 = sm_pool.tile([HD, SQ], BF16, name="a1")
                        nc.vector.tensor_mul(
                            out=a1[:], in0=raw1[0:HD, :], in1=bc1[:]
                        )
                        nc.gpsimd.dma_start(attp_sb[m][HD:128, :], a1[:])
                    else:
                        # last pair: shortest chain to phase D. ACT lifts
                        # the denom rows to SBUF (same partition), DVE
                        # reciprocates, a matmul against a ones row
                        # broadcasts 1/d down the partitions; DVE
                        # evacuates raw attn in parallel and multiplies
                        # SBUF x PSUM. No DMA hops.
                        raw0 = sm_pool.tile([HD + 1, SQ], FP32, name="raw0")
                        nc.vector.tensor_copy(out=raw0[:], in_=at0[:])
                        raw1 = sm_pool.tile([HD + 1, SQ], FP32, name="raw1")
                        nc.vector.tensor_copy(out=raw1[:], in_=at1[:])
                        rb0 = sm_pool.tile([HD + 1, SQ], BF16, name="rb0")
                        rb1 = sm_pool.tile([HD + 1, SQ], BF16, name="rb1")
                        with nc.allow_low_precision(
                            reason="softmax denom reciprocal broadcast in bf16"
                        ):
                            nc.vector.reciprocal(
                                rb0[HD : HD + 1, :], raw0[HD : HD + 1, :]
                            )
                            nc.vector.reciprocal(
                                rb1[HD : HD + 1, :], raw1[HD : HD + 1, :]
                            )
                        # phase-D group 0's pairs-0..4 accumulation runs on
                        # the PE while DVE produces the reciprocals above
                        d0_pss = [
                            pt_psp.tile([128, 1024], FP32, name="pt")[:, 0:384]
                            for _ in range(2)
                        ]
                        for mm in range(MP - 1):
                            for ns in range(2):
                                nc.tensor.matmul(
                                    d0_pss[ns],
                                    attp_sb[mm][:, 0:128],
                                    wo_sb[:, mm, 384 * ns : 384 * (ns + 1)],
                                    start=(mm == 0),
                                    stop=False,
                                )
                        bc_ps = pt_psp.tile([128, 1024], FP32, name="pt")
                        nc.tensor.matmul(
                            bc_ps[0:HD, 0:SQ],
                            ones_f[HD : HD + 1, :],
                            rb0[HD : HD + 1, :],
                            start=True,
                            stop=True,
                        )
                        nc.tensor.matmul(
                            bc_ps[0:HD, SQ : 2 * SQ],
                            ones_f[HD : HD + 1, :],
                            rb1[HD : HD + 1, :],
                            start=True,
                            stop=True,
                        )
                        nc.vector.tensor_mul(
                            out=att5u[:],
                            in0=raw0[0:HD, :],
                            in1=bc_ps[0:HD, 0:SQ],
                        )
                        nc.vector.tensor_mul(
                            out=att5l[:],
                            in0=raw1[0:HD, :],
                            in1=bc_ps[0:HD, SQ : 2 * SQ],
                        )

                # ---- phase D (same scope; PSUM reuses the pt slots) ----
                for i in range(SQ // 128):
                    isl = slice(128 * i, 128 * (i + 1))
                    if i == 0:
                        pss = d0_pss
                    else:
                        pss = [
                            pt_psp.tile([128, 1024], FP32, name="pt")[:, 0:384]
                            for _ in range(2)
                        ]
                        for mm in range(MP - 1):
                            for ns in range(2):
                                nc.tensor.matmul(
                                    pss[ns],
                                    attp_sb[mm][:, isl],
                                    wo_sb[:, mm, 384 * ns : 384 * (ns + 1)],
                                    start=(mm == 0),
                                    stop=False,
                                )
                    for ns in range(2):
                        nc.tensor.matmul(
                            pss[ns],
                            att5u[:, isl],
                            wo_sb[0:HD, MP - 1, 384 * ns : 384 * (ns + 1)],
                            start=False,
                            stop=False,
                        )
                    for ns in range(2):
                        nc.tensor.matmul(
                            pss[ns],
                            att5l[:, isl],
                            wo5_lo[:, 384 * ns : 384 * (ns + 1)],
                            start=False,
                            stop=True,
                        )
                    for ns in range(2):
                        o_ev = opool.tile([128, 384], FP32, name="o_ev")
                        nc.vector.tensor_add(
                            out=o_ev[:],
                            in0=pss[ns],
                            in1=bo_bc[:, 384 * ns : 384 * (ns + 1)],
                        )
                        nc.scalar.dma_start(
                            out[isl, 384 * ns : 384 * (ns + 1)], o_ev[:]
                        )

    nc.finalize()
    return nc


_NC_CACHE = None


def _get_nc():
    global _NC_CACHE
    if _NC_CACHE is None:
        _NC_CACHE = build_nc()
    return _NC_CACHE


def make_in_maps(hidden_states, Wq, Wk, Wv, bq, bk, bv, Wo, bo):
    x = np.asarray(hidden_states, dtype=np.float32)[0]  # [S, D]
    scale = 1.0 / np.sqrt(np.float32(HD))

    xT = np.ascontiguousarray(x.T)  # [D, S]
    xt_r = np.ascontiguousarray(
        xT.reshape(KC, 128, S).transpose(1, 0, 2).astype(ml_dtypes.bfloat16)
    )
    wq_all = np.ascontiguousarray(
        (np.asarray(Wq) * scale).transpose(1, 0, 2).reshape(D, D).astype(np.float32)
    )
    wk_all = np.ascontiguousarray(
        np.asarray(Wk).transpose(1, 0, 2).reshape(D, D).astype(np.float32)
    )
    wv_all = np.ascontiguousarray(
        np.asarray(Wv).transpose(1, 0, 2).reshape(D, D).astype(np.float32)
    )
    wo_r = np.ascontiguousarray(
        np.asarray(Wo, dtype=np.float32)
        .reshape(MP, 128, D)
        .transpose(1, 0, 2)
        .astype(ml_dtypes.bfloat16)
    )  # [128, MP, D]
    bq_r = np.ascontiguousarray(
        (np.asarray(bq) * scale).reshape(D).reshape(MP, 128).T.astype(np.float32)
    )  # [128, MP]
    bk_r = np.ascontiguousarray(
        np.asarray(bk, dtype=np.float32).reshape(D).reshape(MP, 128).T
    )
    bv_r = np.asarray(bv, dtype=np.float32).reshape(1, D)
    bo_r = np.asarray(bo, dtype=np.float32).reshape(1, D)

    def karr(w):  # [D, D] -> [128, KC, D] bf16
        return np.ascontiguousarray(
            w.reshape(KC, 128, D).transpose(1, 0, 2).astype(ml_dtypes.bfloat16)
        )

    wq_all, wk_all, wv_all = karr(wq_all), karr(wk_all), karr(wv_all)
    in_maps = []
    for c in range(N_CORES):
        in_maps.append(
            {
                "xct": np.ascontiguousarray(xt_r[:, :, SQ * c : SQ * (c + 1)]),
                "xt": xt_r,
                "wq": wq_all,
                "wk": wk_all,
                "wv": wv_all,
                "wo": wo_r,
                "bq": bq_r,
                "bk": bk_r,
                "bv": bv_r,
                "bo": bo_r,
            }
        )
    return in_maps


def kernel(hidden_states, Wq, Wk, Wv, bq, bk, bv, Wo, bo):
    in_maps = make_in_maps(hidden_states, Wq, Wk, Wv, bq, bk, bv, Wo, bo)
    nc = _get_nc()
    last_err = None
    for _attempt in range(3):
        try:
            res = run_bass_kernel_spmd(nc, in_maps, list(range(N_CORES)))
            break
        except Exception as e:  # transient NRT_EXEC_UNIT_UNRECOVERABLE seen rarely
            last_err = e
            import time

            time.sleep(2.0)
    else:
        raise last_err
    outs = [res.results[c]["out"] for c in range(N_CORES)]
    return np.concatenate(outs, axis=0)[None, :, :].astype(np.float32)
